# revision 37
# baseline (speedup 1.0000x reference)
"""BailingMoE linear attention (lightning attention) on 8 trn2 NeuronCores.

Tensor-parallel over heads: 2 heads per core. Full inputs in, full output out.
fp16 matmul operands (fp32 PSUM accumulation) throughout; norm math fp32.
Per core: qkv+g projections (q,k,g in [chan,seq] layout, v in [seq,chan]),
per-head RMSNorm (Pool partition_broadcast for the rstd bcast) + neox RoPE
(DVE partition-offset half-swap), chunked linear attention with decayed kv
state in SBUF (causally-trimmed score matmuls), group RMSNorm + sigmoid gate,
then a 4-piece pipelined AllToAll (fp16) with chunk-interleaved sequence
sharding so the dense projection overlaps the attention main loop.
Core j's output rows are global chunks {j, j+8, j+16, j+24}.
"""
import math

import numpy as np

S = 8192
HID = 2048
H = 16
D = 128
BLK = 256
GROUPS = 8
EPS = 1e-5
ROPE_THETA = 600000.0
SCALE = D ** -0.5
N_CORES = 8
HPC = H // N_CORES          # heads per core = 2
CPC = HPC * D               # channels per core = 256
KT = 16                     # contraction tiles (2048 / 128)
SEQ_G = 512                 # seq per projection group
NG = S // SEQ_G             # 16 groups
NC_CH = S // BLK            # 32 chunks
CPG = SEQ_G // BLK          # chunks per group = 2
NP = 4                      # AllToAll pieces
CHP = NC_CH // NP           # chunks per piece = 8

_cache = {}


def _build_slopes():
    start = 2.0 ** (-(2.0 ** (-(math.log2(H) - 3.0))))
    slopes = np.array([start * start ** i for i in range(H)], dtype=np.float32)
    return slopes * np.float32(1.0 - 0.0 / (20 - 1) + 1e-5)


def _build_program():
    import concourse.bacc as bacc
    import concourse.tile as tile
    import concourse.mybir as mybir
    from contextlib import ExitStack

    dt = mybir.dt
    AF = mybir.ActivationFunctionType
    OP = mybir.AluOpType
    f16 = dt.float16
    f32 = dt.float32
    f32r = dt.float32r

    nc = bacc.Bacc("TRN2", target_bir_lowering=False, debug=False,
                   num_devices=N_CORES)

    def din(name, shape, dtype=f32):
        return nc.dram_tensor(name, shape, dtype, kind="ExternalInput").ap()

    hsT = din("hsT", [HID, S], f16)
    wB = din("wB", [HID, 768], f16)        # cols: q(256) k(256) g(256)
    wv = din("wv", [HID, 256], f16)
    dwT = din("dwT", [HID, HID], f16)
    cosf = din("cosf", [128, S], f16)       # [cos; cos]
    sinb = din("sinb", [128, S], f16)       # [+sin; -sin]
    qb_d = din("qb", [128, HPC])
    kb_d = din("kb", [128, HPC])
    vbb_d = din("vbb", [128, 256], f16)
    qdec_d = din("qdec", [128, HPC, BLK], f16)
    kdec_d = din("kdec", [128, HPC, BLK], f16)
    dg0_d = din("dg0", [128, HPC, BLK], f16)
    dg1_d = din("dg1", [128, HPC, 128], f16)
    qnw_d = din("qnw", [128, 1])
    knw_d = din("knw", [128, 1])
    gnw_d = din("gnw", [128, HPC])
    blk_d = din("blkdec", [128, HPC])
    ones16_d = din("ones16", [128, 1], f16)
    idm16_d = din("idm16", [128, 128], f16)
    zkv_d = din("zkv", [128, 128], f32r)

    out_d = nc.dram_tensor("out", [NP * BLK, HID], f32,
                           kind="ExternalOutput").ap()

    with tile.TileContext(nc) as tc:
        ctx = ExitStack()
        consts = ctx.enter_context(tc.tile_pool(name="consts", bufs=1))
        wpool = ctx.enter_context(tc.tile_pool(name="wpool", bufs=1))
        dramp = ctx.enter_context(tc.tile_pool(name="dramp", bufs=1,
                                               space="DRAM"))
        hkp = ctx.enter_context(tc.tile_pool(name="hkp", bufs=32))
        tabp = ctx.enter_context(tc.tile_pool(name="tabp", bufs=2))
        evp = ctx.enter_context(tc.tile_pool(name="evp", bufs=2))
        xrp = ctx.enter_context(tc.tile_pool(name="xrp", bufs=1))
        natp = ctx.enter_context(tc.tile_pool(name="natp", bufs=1))
        attp = ctx.enter_context(tc.tile_pool(name="attp", bufs=2))
        kvpl = ctx.enter_context(tc.tile_pool(name="kvpl", bufs=1))
        yp = ctx.enter_context(tc.tile_pool(name="yp", bufs=3))
        dyp = ctx.enter_context(tc.tile_pool(name="dyp", bufs=16))
        dop = ctx.enter_context(tc.tile_pool(name="dop", bufs=2))
        # PSUM: psp(2) + psa(4) + aux(2) = 8 banks
        psp = ctx.enter_context(tc.tile_pool(name="psp", bufs=2, space="PSUM"))
        psa = ctx.enter_context(tc.tile_pool(name="psa", bufs=4, space="PSUM"))
        aux = ctx.enter_context(tc.tile_pool(name="aux", bufs=2, space="PSUM"))

        y_send = [dramp.tile([N_CORES, CPC, BLK], f16, name=f"y_send{p}",
                             tag=f"y_send{p}") for p in range(NP)]
        y_recv = [dramp.tile([N_CORES, CPC, BLK], f16, name=f"y_recv{p}",
                             tag=f"y_recv{p}") for p in range(NP)]

        def cload(name, ap_src, shape, dtype=f32):
            t = consts.tile(shape, dtype, name=name, tag=name)
            nc.sync.dma_start(out=t[:], in_=ap_src)
            return t

        # group-0 activations + weights first so the PE can start ASAP
        hk_cache = {}

        def emit_hk(g):
            s0 = g * SEQ_G
            hk = []
            for t in range(KT):
                hkt = hkp.tile([128, SEQ_G], f16, name=f"hk{g}_{t}", tag="hk")
                nc.sync.dma_start(out=hkt[:],
                                  in_=hsT[t * 128:(t + 1) * 128, s0:s0 + SEQ_G])
                hk.append(hkt)
            cos_g = tabp.tile([128, SEQ_G], f16, name=f"cos{g}", tag="cos")
            nc.sync.dma_start(out=cos_g[:], in_=cosf[:, s0:s0 + SEQ_G])
            sin_g = tabp.tile([128, SEQ_G], f16, name=f"sin{g}", tag="sin")
            nc.sync.dma_start(out=sin_g[:], in_=sinb[:, s0:s0 + SEQ_G])
            hk_cache[g] = (hk, cos_g, sin_g)

        wB_sb = wpool.tile([128, KT, 768], f16, name="wB_sb")
        nc.sync.dma_start(
            out=wB_sb[:, :, 0:128],
            in_=wB[:, 0:128].rearrange("(t p) c -> p t c", p=128))
        emit_hk(0)
        wv_sb = wpool.tile([128, KT, 256], f16, name="wv_sb")
        nc.sync.dma_start(out=wv_sb[:],
                          in_=wv.rearrange("(t p) c -> p t c", p=128))
        for ci in range(1, 6):
            nc.sync.dma_start(
                out=wB_sb[:, :, ci * 128:(ci + 1) * 128],
                in_=wB[:, ci * 128:(ci + 1) * 128].rearrange(
                    "(t p) c -> p t c", p=128))

        qb = cload("qb_s", qb_d, [128, HPC])
        kb = cload("kb_s", kb_d, [128, HPC])
        vbb = cload("vbb_s", vbb_d, [128, 256], f16)
        qdec = cload("qdec_s", qdec_d, [128, HPC, BLK], f16)
        kdec = cload("kdec_s", kdec_d, [128, HPC, BLK], f16)
        dg0 = cload("dg0_s", dg0_d, [128, HPC, BLK], f16)
        dg1 = cload("dg1_s", dg1_d, [128, HPC, 128], f16)
        qnw = cload("qnw_s", qnw_d, [128, 1])
        knw = cload("knw_s", knw_d, [128, 1])
        gnw = cload("gnw_s", gnw_d, [128, HPC])
        blkd = cload("blkd_s", blk_d, [128, HPC])
        ones16 = cload("ones16_s", ones16_d, [128, 1], f16)
        idm16 = cload("idm16_s", idm16_d, [128, 128], f16)
        epsb = consts.tile([1, 1], f32, name="epsb", tag="epsb")
        nc.vector.memset(epsb[:], EPS)
        sclb = consts.tile([1, 1], f32, name="sclb", tag="sclb")
        nc.vector.memset(sclb[:], math.log(SCALE))
        zerb = consts.tile([1, 1], f32, name="zerb", tag="zerb")
        nc.vector.memset(zerb[:], 0.0)

        dwts = [wpool.tile([128, HID], f16, name=f"dwt{t}", tag=f"dw{t}")
                for t in range(KT)]

        # persistent kv state, ping-pong per head
        kv_sb = [[kvpl.tile([128, 128], f32r, name=f"kv{h}_{i}",
                            tag=f"kv{h}_{i}") for i in range(2)]
                 for h in range(HPC)]
        for h in range(HPC):
            nc.sync.dma_start(out=kv_sb[h][0][:], in_=zkv_d)

        state = {}

        def emit_proj_group(g):
            if g not in hk_cache:
                emit_hk(g)
            hk, cos_g, sin_g = hk_cache.pop(g)
            if g + 1 < NG:
                emit_hk(g + 1)  # prefetch next group's activations

            xr_t, sig_t = [], []

            def norm_tail(ci, xb):
                is_q = ci < 2
                sq = evp.tile([128, SEQ_G], f16, name=f"sq{g}_{ci}", tag="sq", bufs=2)
                nc.scalar.activation(sq[:], xb[:], AF.Square)
                ssq = aux.tile([1, SEQ_G], f32, name=f"ssq{g}_{ci}",
                               tag="aux")
                nc.tensor.matmul(ssq[:], ones16[:], sq[:],
                                 start=True, stop=True)
                lnt = evp.tile([1, SEQ_G], f32, name=f"ln{g}_{ci}", tag="ln")
                nc.scalar.activation(lnt[:], ssq[:], AF.Ln,
                                     bias=epsb[:], scale=1.0 / D)
                rstd = evp.tile([1, SEQ_G], f16, name=f"rstd{g}_{ci}",
                                tag="rstd")
                nc.scalar.activation(rstd[:], lnt[:], AF.Exp, scale=-0.5,
                                     bias=sclb[:] if is_q else zerb[:])
                rbc = evp.tile([128, SEQ_G], f16, name=f"rbc{g}_{ci}",
                               tag="rbc", bufs=2)
                nc.gpsimd.partition_broadcast(rbc[:], rstd[:])
                xn = evp.tile([128, SEQ_G], f16, name=f"xn{g}_{ci}", tag="xn", bufs=1)
                nc.vector.scalar_tensor_tensor(
                    out=xn[:], in0=xb[:], scalar=qnw[:] if is_q else knw[:],
                    in1=rbc[:], op0=OP.mult, op1=OP.mult)
                # rope: xr = xn*cos + halfswap(xn)*sin_signed
                m2 = evp.tile([128, SEQ_G], f16, name=f"m2{g}_{ci}", tag="m2",
                              bufs=2)
                nc.vector.tensor_tensor(out=m2[0:64, :], in0=xn[64:128, :],
                                        in1=sin_g[64:128, :], op=OP.mult)
                nc.vector.tensor_tensor(out=m2[64:128, :], in0=xn[0:64, :],
                                        in1=sin_g[0:64, :], op=OP.mult)
                xr = xrp.tile([128, SEQ_G], f16, name=f"xr{g}_{ci}",
                              tag=f"xr{ci}", bufs=2)
                nc.vector.tensor_tensor(out=xr[:], in0=xn[:], in1=cos_g[:],
                                        op=OP.mult)
                nc.vector.tensor_tensor(out=xr[:], in0=xr[:], in1=m2[:],
                                        op=OP.add)
                xr_t.append(xr)

            def g_tail(ci, eg):
                nc.vector.tensor_scalar_add(eg[:], eg[:], 1.0)
                sig = xrp.tile([128, SEQ_G], f16, name=f"sig{g}_{ci}",
                               tag=f"sig{ci}", bufs=2)
                with nc.allow_low_precision(reason="sigmoid in [0,1]"):
                    nc.vector.reciprocal(sig[:], eg[:])
                sig_t.append(sig)

            # q/k/g accumulations, each norm tail lagged two accs behind so
            # the PE never waits on the DVE/ACT chain
            pend = []
            for ci in range(6):  # 0,1=q  2,3=k  4,5=g
                acc = psp.tile([128, SEQ_G], f32, name=f"acc{g}_{ci}",
                               tag="ps")
                for t in range(KT):
                    nc.tensor.matmul(acc[:],
                                     wB_sb[:, t, ci * 128:(ci + 1) * 128],
                                     hk[t][:], start=(t == 0),
                                     stop=(t == KT - 1))
                if ci < 4:
                    xb = evp.tile([128, SEQ_G], f16, name=f"xb{g}_{ci}",
                                  tag="xb", bufs=3)
                    nc.vector.tensor_scalar_add(
                        xb[:], acc[:], (qb if ci < 2 else kb)[:, ci % 2:ci % 2 + 1])
                    mine = ("n", ci, xb)
                else:
                    eg = evp.tile([128, SEQ_G], f16, name=f"eg{g}_{ci}",
                                  tag="eg")
                    nc.scalar.activation(eg[:], acc[:], AF.Exp, scale=-1.0)
                    mine = ("g", ci, eg)
                pend.append(mine)
                if len(pend) > 2:
                    t0 = pend.pop(0)
                    (norm_tail if t0[0] == "n" else g_tail)(t0[1], t0[2])
            v_nat = []
            v_accs = []
            for s2 in range(2):
                accv = psp.tile([128, SEQ_G], f32, name=f"accv{g}_{s2}",
                                tag="ps")
                for half in range(2):
                    st = s2 * 2 + half
                    for t in range(KT):
                        nc.tensor.matmul(
                            accv[:, half * 256:(half + 1) * 256],
                            hk[t][:, st * 128:(st + 1) * 128],
                            wv_sb[:, t, :],
                            start=(t == 0), stop=(t == KT - 1))
                v_accs.append(accv)
                if pend:
                    t0 = pend.pop(0)
                    (norm_tail if t0[0] == "n" else g_tail)(t0[1], t0[2])
            for s2 in range(2):
                for half in range(2):
                    vn = natp.tile([128, 256], f16, name=f"vn{g}_{s2}_{half}",
                                   tag="vn", bufs=8)
                    nc.vector.scalar_tensor_tensor(
                        out=vn[:],
                        in0=v_accs[s2][:, half * 256:(half + 1) * 256],
                        scalar=1.0, in1=vbb[:], op0=OP.mult, op1=OP.add)
                    v_nat.append(vn)

            # decayed k + transposes
            knat = [[None] * CPG for _ in range(HPC)]
            for h in range(HPC):
                ktil = evp.tile([128, SEQ_G], f16, name=f"ktil{g}_{h}",
                                tag="ktil")
                for cc in range(CPG):
                    nc.vector.tensor_tensor(
                        out=ktil[:, cc * BLK:(cc + 1) * BLK],
                        in0=xr_t[2 + h][:, cc * BLK:(cc + 1) * BLK],
                        in1=kdec[:, h, :], op=OP.mult)
                for cc in range(CPG):
                    if g == NG - 1 and cc == CPG - 1:
                        continue  # last chunk's kv update is skipped
                    kn_list = []
                    for j in range(2):
                        tp = aux.tile([128, 128], f16, name=f"tp{g}_{h}_{cc}_{j}",
                                      tag="aux")
                        nc.tensor.transpose(
                            tp[:],
                            ktil[:, cc * BLK + j * 128:cc * BLK + (j + 1) * 128],
                            idm16[:])
                        kn = natp.tile([128, 128], f16,
                                       name=f"kn{g}_{h}_{cc}_{j}", tag="kn",
                                       bufs=16)
                        nc.scalar.activation(kn[:], tp[:], AF.Copy)
                        kn_list.append(kn)
                    knat[h][cc] = kn_list

            state[g] = (xr_t, sig_t, v_nat, knat)

        def emit_attn_group(ag):
            xr_t, sig_t, v_nat, knat = state.pop(ag)
            chunk_tails = []
            for cc in range(CPG):
                ch = ag * CPG + cc
                p, dest = ch // CHP, ch % CHP
                kq_l, kqd_l, qt_l, ob_l, sqh_l = [], [], [], [], []
                for h in range(HPC):
                    qr = xr_t[h][:, cc * BLK:(cc + 1) * BLK]
                    kr = xr_t[2 + h]
                    kq = psa.tile([128, BLK + 128], f32, name=f"kq{ch}_{h}",
                                  tag="psa")
                    nc.tensor.matmul(
                        kq[:, 0:BLK],
                        kr[:, cc * BLK:cc * BLK + 128], qr,
                        start=True, stop=True)
                    nc.tensor.matmul(
                        kq[:, BLK:BLK + 128],
                        kr[:, cc * BLK + 128:cc * BLK + 256],
                        xr_t[h][:, cc * BLK + 128:cc * BLK + 256],
                        start=True, stop=True)
                    kq_l.append(kq)
                for h in range(HPC):
                    kqd0 = attp.tile([128, BLK], f16, name=f"kqd0_{ch}_{h}",
                                     tag="kqd0")
                    nc.vector.tensor_tensor(out=kqd0[:], in0=kq_l[h][:, 0:BLK],
                                            in1=dg0[:, h, :], op=OP.mult)
                    kqd1 = attp.tile([128, 128], f16, name=f"kqd1_{ch}_{h}",
                                     tag="kqd1")
                    nc.vector.tensor_tensor(out=kqd1[:],
                                            in0=kq_l[h][:, BLK:BLK + 128],
                                            in1=dg1[:, h, :], op=OP.mult)
                    kqd_l.append((kqd0, kqd1))
                    qt = attp.tile([128, BLK], f32r, name=f"qt{ch}_{h}",
                                   tag="qt")
                    nc.vector.tensor_tensor(
                        out=qt[:], in0=xr_t[h][:, cc * BLK:(cc + 1) * BLK],
                        in1=qdec[:, h, :], op=OP.mult)
                    qt_l.append(qt)
                for h in range(HPC):
                    kv_cur = kv_sb[h][ch % 2]
                    ops = psa.tile([128, BLK], f32, name=f"ops{ch}_{h}",
                                   tag="psa")
                    nc.tensor.matmul(ops[:],
                                     v_nat[cc * 2][:, h * 128:(h + 1) * 128],
                                     kqd_l[h][0][:], start=True, stop=False)
                    nc.tensor.matmul(ops[:, 128:BLK],
                                     v_nat[cc * 2 + 1][:, h * 128:(h + 1) * 128],
                                     kqd_l[h][1][:], start=False, stop=False)
                    nc.tensor.matmul(ops[:], kv_cur[:], qt_l[h][:],
                                     start=False, stop=True)
                    ob = attp.tile([128, BLK], f16, name=f"ob{ch}_{h}",
                                   tag="ob", bufs=6)
                    nc.scalar.activation(ob[:], ops[:], AF.Copy)
                    ob_l.append(ob)
                    sqh = attp.tile([128, BLK], f16, name=f"sqh{ch}_{h}",
                                    tag="sqh", bufs=6)
                    nc.scalar.activation(sqh[:], ob[:], AF.Square)
                    sqh_l.append(sqh)
                    if ch < NC_CH - 1:
                        kvp_ps = psa.tile([128, 128], f32, name=f"kvp{ch}_{h}",
                                          tag="psa")
                        for j in range(2):
                            nc.tensor.matmul(
                                kvp_ps[:], knat[h][cc][j][:],
                                v_nat[cc * 2 + j][:, h * 128:(h + 1) * 128],
                                start=(j == 0), stop=(j == 1))
                        nc.vector.scalar_tensor_tensor(
                            out=kv_sb[h][(ch + 1) % 2][:], in0=kv_cur[:],
                            scalar=blkd[:, h:h + 1], in1=kvp_ps[:],
                            op0=OP.mult, op1=OP.add)

                chunk_tails.append((ch, cc, p, dest, ob_l, sqh_l))
                if ag == NG - 1:
                    emit_gnorm_tails(chunk_tails, sig_t)
                    chunk_tails = []

            # groupnorm tails, deferred so the PE never waits on ACT squares
            emit_gnorm_tails(chunk_tails, sig_t)

        def emit_gnorm_tails(chunk_tails, sig_t):
            for ch, cc, p, dest, ob_l, sqh_l in chunk_tails:
                gssq = aux.tile([1, BLK], f32, name=f"gssq{ch}", tag="aux")
                for h in range(HPC):
                    nc.tensor.matmul(gssq[:], ones16[:], sqh_l[h][:],
                                     start=(h == 0), stop=(h == HPC - 1))
                glt = attp.tile([1, BLK], f32, name=f"glt{ch}", tag="glt")
                nc.scalar.activation(glt[:], gssq[:], AF.Ln,
                                     bias=epsb[:], scale=1.0 / CPC)
                grstd = attp.tile([1, BLK], f16, name=f"grstd{ch}",
                                  tag="grstd")
                nc.scalar.activation(grstd[:], glt[:], AF.Exp, scale=-0.5)
                grbc = attp.tile([128, BLK], f16, name=f"grbc{ch}",
                                 tag="grbc")
                nc.gpsimd.partition_broadcast(grbc[:], grstd[:])
                for h in range(HPC):
                    y1 = yp.tile([128, BLK], f16, name=f"y1{ch}_{h}",
                                 tag="y1", bufs=2)
                    nc.vector.scalar_tensor_tensor(
                        out=y1[:], in0=ob_l[h][:], scalar=gnw[:, h:h + 1],
                        in1=grbc[:], op0=OP.mult, op1=OP.mult)
                    y2 = yp.tile([128, BLK], f16, name=f"y2{ch}_{h}",
                                 tag="y2")
                    nc.vector.tensor_tensor(
                        out=y2[:], in0=y1[:],
                        in1=sig_t[h][:, cc * BLK:(cc + 1) * BLK], op=OP.mult)
                    nc.sync.dma_start(
                        out=y_send[p][dest, h * 128:(h + 1) * 128, :],
                        in_=y2[:])

        def emit_a2a(p):
            import concourse.mybir as mybir
            nc.gpsimd.collective_compute(
                "AllToAll", mybir.AluOpType.bypass,
                replica_groups=[list(range(N_CORES))],
                ins=[y_send[p][:].opt()],
                outs=[y_recv[p][:].opt()],
            )

        yt_cache = {}

        def emit_dense_load(p):
            yt = []
            for t in range(KT):
                ytt = dyp.tile([128, BLK], f16, name=f"yt{p}_{t}", tag="yt")
                nc.sync.dma_start(
                    out=ytt[:],
                    in_=y_recv[p][t // 2, (t % 2) * 128:(t % 2) * 128 + 128, :])
                yt.append(ytt)
            yt_cache[p] = yt

        def emit_dense_mm(p):
            yt = yt_cache.pop(p)
            for st in range(2):
                for hq in range(4):
                    acc = psp.tile([128, 512], f32, name=f"dacc{p}_{st}_{hq}",
                                   tag="ps")
                    for t in range(KT):
                        nc.tensor.matmul(
                            acc[:], yt[t][:, st * 128:(st + 1) * 128],
                            dwts[t][:, hq * 512:(hq + 1) * 512],
                            start=(t == 0), stop=(t == KT - 1))
                    ot = dop.tile([128, 512], f32, name=f"ot{p}_{st}_{hq}",
                                  tag="ot")
                    nc.scalar.activation(ot[:], acc[:], AF.Copy)
                    nc.sync.dma_start(
                        out=out_d[p * BLK + st * 128:p * BLK + (st + 1) * 128,
                                  hq * 512:(hq + 1) * 512],
                        in_=ot[:])

        for g in range(NG + 1):
            if g < NG:
                emit_proj_group(g)
            if 1 <= g <= 4:
                for t in range((g - 1) * 4, g * 4):
                    nc.sync.dma_start(out=dwts[t][:],
                                      in_=dwT[t * 128:(t + 1) * 128, :])
            if g >= 1:
                ag = g - 1
                emit_attn_group(ag)
                if ag % 4 == 3:
                    p = ag // 4
                    emit_a2a(p)
                    if p >= 1:
                        emit_dense_mm(p - 1)
                    emit_dense_load(p)
        # dense piece 3 fills the PE while AllToAll 3 completes
        emit_dense_mm(3)

        ctx.close()

    nc.compile()
    return nc


def _stage(hidden_states, positions, qkv_w, qkv_b, q_norm_w, k_norm_w,
           g_w, g_norm_w, dense_w):
    f32 = np.float32
    f16 = np.float16
    hidden_states = np.asarray(hidden_states, dtype=f32)
    positions = np.asarray(positions)
    qkv_w = np.asarray(qkv_w, dtype=f32)
    qkv_b = np.asarray(qkv_b, dtype=f32)
    q_norm_w = np.asarray(q_norm_w, dtype=f32)
    k_norm_w = np.asarray(k_norm_w, dtype=f32)
    g_w = np.asarray(g_w, dtype=f32)
    g_norm_w = np.asarray(g_norm_w, dtype=f32)
    dense_w = np.asarray(dense_w, dtype=f32)
    slopes = _build_slopes()

    hsT = np.ascontiguousarray(hidden_states.T).astype(f16)

    inv_freq = 1.0 / (ROPE_THETA ** (np.arange(0, D, 2, dtype=f32) / D))
    freqs = positions.astype(f32)[:, None] * inv_freq[None, :]  # [S, 64]
    cos = np.cos(freqs).T.astype(f32)     # [64, S]
    sin = np.sin(freqs).T.astype(f32)
    cosf = np.ascontiguousarray(np.concatenate([cos, cos], axis=0)).astype(f16)
    sinb = np.ascontiguousarray(np.concatenate([sin, -sin], axis=0)).astype(f16)

    idx = np.arange(BLK, dtype=f32)
    dwT = np.ascontiguousarray(dense_w.T).astype(f16)
    ones16 = np.ones((128, 1), dtype=f16)
    idm16 = np.eye(128, dtype=f16)
    qnw = q_norm_w.reshape(128, 1).copy()
    knw = k_norm_w.reshape(128, 1).copy()

    in_maps = []
    for j in range(N_CORES):
        c0 = j * CPC
        wBm = np.empty((HID, 768), dtype=f16)
        wBm[:, 0:256] = qkv_w[c0:c0 + CPC, :].T
        wBm[:, 256:512] = qkv_w[HID + c0:HID + c0 + CPC, :].T
        wBm[:, 512:768] = g_w[c0:c0 + CPC, :].T
        wvm = np.ascontiguousarray(
            qkv_w[2 * HID + c0:2 * HID + c0 + CPC, :].T).astype(f16)
        qbm = np.ascontiguousarray(
            qkv_b[c0:c0 + CPC].reshape(HPC, 128).T)
        kbm = np.ascontiguousarray(
            qkv_b[HID + c0:HID + c0 + CPC].reshape(HPC, 128).T)
        vbb = np.ascontiguousarray(np.broadcast_to(
            qkv_b[2 * HID + c0:2 * HID + c0 + CPC][None, :],
            (128, 256))).astype(f16)

        sl = slopes[j * HPC:(j + 1) * HPC]  # [HPC]
        qdec = np.exp(-sl[:, None] * (idx + 1.0)[None, :]).astype(f32)
        qdec = np.ascontiguousarray(
            np.broadcast_to(qdec[None, :, :], (128, HPC, BLK))).astype(f16)
        kd = np.exp(-sl[:, None] * (BLK - 1.0 - idx)[None, :]).astype(f32)
        kdecm = np.ascontiguousarray(
            np.broadcast_to(kd[None, :, :], (128, HPC, BLK))).astype(f16)
        dif = idx[:, None] - idx[None, :]           # [i, j]
        dg0 = np.zeros((128, HPC, BLK), dtype=f16)
        dg1 = np.zeros((128, HPC, 128), dtype=f16)
        for hh in range(HPC):
            dd = np.where(
                dif >= 0,
                np.exp(-sl[hh] * np.where(dif >= 0, dif, 0.0)),
                0.0).astype(f32)                    # [i, j]
            ddT = dd.T                               # [j, i]
            dg0[:, hh, :] = ddT[0:128, :]
            dg1[:, hh, :] = ddT[128:256, 128:256]
        blkdec = np.ascontiguousarray(np.broadcast_to(
            np.exp(-sl * BLK).astype(f32)[None, :], (128, HPC)))
        gnwm = np.ascontiguousarray(g_norm_w[c0:c0 + CPC].reshape(HPC, 128).T)

        in_maps.append({
            "hsT": hsT, "wB": wBm, "wv": wvm, "dwT": dwT,
            "cosf": cosf, "sinb": sinb,
            "qb": qbm, "kb": kbm, "vbb": vbb,
            "qdec": qdec, "kdec": kdecm, "dg0": dg0, "dg1": dg1,
            "qnw": qnw, "knw": knw, "gnw": gnwm, "blkdec": blkdec,
            "ones16": ones16, "idm16": idm16,
            "zkv": np.zeros((128, 128), dtype=f32),
        })
    return in_maps


def assemble(outs):
    """outs: list of 8 per-core [NP*BLK, HID] arrays -> [S, HID]."""
    full = np.empty((S, HID), dtype=np.float32)
    for j in range(N_CORES):
        for p in range(NP):
            ch = p * CHP + j
            full[ch * BLK:(ch + 1) * BLK] = outs[j][p * BLK:(p + 1) * BLK]
    return full


def kernel(**inputs):
    from concourse.bass_utils import run_bass_kernel_spmd

    if "nc" not in _cache:
        _cache["nc"] = _build_program()
    nc = _cache["nc"]
    in_maps = _stage(**inputs)
    res = run_bass_kernel_spmd(nc, in_maps, list(range(N_CORES)))
    return assemble([res.results[j]["out"] for j in range(N_CORES)])
